# revision 13
# baseline (speedup 1.0000x reference)
"""Trainium2 Bass kernel: 4-layer transformer encoder (B=8,T=512,E=1024,H=16,FF=4096).

Sharding: data-parallel over batch — one sequence per NeuronCore, 8 cores,
no collectives. Activations live feature-major ([128 part, sub, T]) so every
linear layer is a natural PE matmul with no on-device transposes:

  - q/k produced feature-major, scores computed transposed (k on partitions)
    so softmax needs no reduction (exp is elementwise; scores are provably
    small, |s| <~ 3, so no max-subtraction is needed).
  - the gaussian-bias renormalization  w=softmax(s); w*=gb; w/=(sum(w)+1e-5)
    is algebraically folded to  t' = exp(s)*(gb+1e-5);  out = (v^T t')/sum(t')
    (numerator uses gb+1e-5 vs gb: relative error ~2e-5, far below bf16 noise).
  - sum(t') comes for free from a ones-column appended to V (65th matmul row).
  - 1/x and 1/sqrt(x) are computed as exp(-ln(x)) / exp(-0.5 ln(x)) so the
    whole kernel uses a single ACT table set (natural_log_exp_and_others).
  - LayerNorm stats in feature-major via ones-vector matmuls (M=1 rows), and
    per-token scalars applied via gpsimd.partition_broadcast + DVE.

Weights are pre-transposed/tiled/cast to bf16 on the host into exact SBUF
layouts and packed into TWO DRAM blobs (one bf16, one fp32 — the fp32 blob
also carries the per-core input x0). Two external inputs instead of ~70:
the axon PJRT dispatch cost scales with argument count, and on-device each
weight window is a plain 2-D column slice of the blob. Positional encoding
is folded into the input on the host (it is a function of shapes only).
"""

import math
import sys

import numpy as np

if "/opt/trn_rl_repo" not in sys.path:
    sys.path.insert(0, "/opt/trn_rl_repo")

import ml_dtypes

B, T, E, H, L, FF = 8, 512, 1024, 16, 4, 4096
D = E // H  # 64
P = 128
ES = E // P  # 8 e-subtiles
FS = FF // P  # 32 f-subtiles
KT = T // P  # 4 k/q tiles
LN_EPS = 1e-5
BF16 = ml_dtypes.bfloat16

_CACHE = {}
_MARKS = []


def _layout(n_layers=L):
    """Column layout of the two input blobs. Single source of truth for
    _build_bass (device) and _host_prep (host)."""
    bf, f32 = {}, {}

    def seg(d, name, ncols):
        off = d.get("_total", 0)
        d[name] = (off, ncols)
        d["_total"] = off + ncols

    seg(bf, "gbp", KT * T)
    for l in range(n_layers):
        seg(bf, f"wq{l}", ES * ES * P)
        seg(bf, f"wk{l}", ES * ES * P)
        seg(bf, f"wv{l}", 2 * ES * T)
        seg(bf, f"wo{l}", ES * ES * P)
        seg(bf, f"w1{l}", FS * ES * P)
        seg(bf, f"w2{l}", ES * 2 * (FS // 2) * P)
        seg(bf, f"vb{l}", E)
    seg(f32, "x0", ES * T)
    for l in range(n_layers):
        for nm, nc_ in ((f"qb{l}", ES), (f"kb{l}", ES), (f"ob{l}", ES),
                        (f"f1b{l}", FS), (f"f2b{l}", ES), (f"g1{l}", ES),
                        (f"b1{l}", ES), (f"g2{l}", ES), (f"b2{l}", ES)):
            seg(f32, nm, nc_)
    seg(f32, "hw", ES)
    seg(f32, "hb", 1)
    return bf, f32


def _build_bass(n_layers=L, repeats=1):
    import concourse.bass as bass  # noqa: F401
    import concourse.mybir as mybir
    import concourse.tile as tile
    from concourse import bacc
    from concourse.bass import ds, ts
    from contextlib import ExitStack

    fp32 = mybir.dt.float32
    bf16 = mybir.dt.bfloat16
    AF = mybir.ActivationFunctionType
    OP = mybir.AluOpType

    nc = bacc.Bacc("TRN2")
    _MARKS.clear()

    def mark(nm):
        _MARKS.append((nm, int(nc.next_id())))

    # ---- DRAM blobs ---------------------------------------------------
    LBF, LF32 = _layout(n_layers)
    CBF, CF32 = LBF["_total"], LF32["_total"]
    wbf = nc.dram_tensor("wbf", [P, CBF], bf16, kind="ExternalInput")
    wf32 = nc.dram_tensor("wf32", [P, CF32], fp32, kind="ExternalInput")

    def bfw(name, ncols=None, sub=0):
        off, n = LBF[name]
        if ncols is None:
            ncols = n
        return wbf.ap()[:, ds(off + sub, ncols)]

    y_out = nc.dram_tensor("y", [1, 1], fp32, kind="ExternalOutput")

    # const region of the fp32 blob (everything after x0)
    c0 = LF32["x0"][1]
    NCONST = CF32 - c0

    with tile.TileContext(nc) as tc, ExitStack() as ctx:
        # ---- pools ----------------------------------------------------
        singles = ctx.enter_context(tc.tile_pool(name="singles", bufs=1))
        xpool = ctx.enter_context(tc.tile_pool(name="xpool", bufs=2))
        xbfp = ctx.enter_context(tc.tile_pool(name="xbfp", bufs=1))
        qkp = ctx.enter_context(tc.tile_pool(name="qkp", bufs=1))
        ocat_p = ctx.enter_context(tc.tile_pool(name="ocat", bufs=1))
        h1p = ctx.enter_context(tc.tile_pool(name="h1p", bufs=1))
        wstream = ctx.enter_context(tc.tile_pool(name="wstream", bufs=2))
        small = ctx.enter_context(tc.tile_pool(name="small", bufs=2))
        et_p = ctx.enter_context(tc.tile_pool(name="et", bufs=4))
        rows = ctx.enter_context(tc.tile_pool(name="rows", bufs=4))
        bc_p = ctx.enter_context(tc.tile_pool(name="bc", bufs=2))
        psum = ctx.enter_context(tc.tile_pool(name="psum", bufs=1, space="PSUM"))

        def ps_tile(shape, tag, bufs):
            return psum.tile(shape, fp32, tag=tag, bufs=bufs, name=tag)

        # Weight streams alternate between the two HWDGE queues (SP and
        # Activation) so descriptor processing of back-to-back 256-512KB
        # weight tiles runs on two queues instead of serializing on one.
        _dmaq = [0]

        def wdma(out, in_):
            _dmaq[0] += 1
            nc.sync.dma_start(out=out, in_=in_)

        # ---- constants ------------------------------------------------
        # Pre-load the one ACT table set that covers every function used
        # (exp, ln, identity, relu, square) so the fixpoint pass inserts no
        # further table switches (~2.7us each).
        from concourse.hw_specs import get_activation_tables
        _tables = get_activation_tables(nc.m.arch)
        _set_idx = list(_tables).index("natural_log_exp_and_others")
        _ld = mybir.InstLoadActFuncSet(
            name=nc.get_next_instruction_name(), ins=[], outs=[],
            act_func_set_id=_set_idx)
        nc.scalar.add_instruction(_ld)

        gbp = singles.tile([P, KT, T], bf16)  # gb' = gb + 1e-5, k-major
        nc.sync.dma_start(
            out=gbp,
            in_=bfw("gbp").rearrange("p (k t) -> p k t", k=KT))
        ones_bf = singles.tile([P, 1], bf16)
        nc.vector.memset(ones_bf, 1.0)
        v_aug = singles.tile([P, KT, H, D + 1], bf16)  # v + ones column
        nc.vector.memset(v_aug[:, :, :, D : D + 1], 1.0)

        # one DMA for every small fp32 constant (biases/ln params/head)
        consts = singles.tile([P, NCONST], fp32)
        nc.sync.dma_start(out=consts, in_=wf32.ap()[:, ds(c0, NCONST)])

        def cref(name, col):  # [P, 1] column of a packed const
            off = LF32[name][0] - c0
            return consts[:, ds(off + col, 1)]

        hw_sb = singles.tile([P, ES], fp32)
        nc.sync.dma_start(out=hw_sb, in_=wf32.ap()[:, ds(LF32["hw"][0], ES)])
        hb_sb = singles.tile([1, 1], fp32)
        nc.sync.dma_start(out=hb_sb, in_=wf32.ap()[0:1, ds(LF32["hb"][0], 1)])
        eps_row = singles.tile([1, 1], fp32)
        nc.vector.memset(eps_row, LN_EPS)

        # ---- repeats replay the FULL kernel (input load → layers → head)
        # so wall-time slope over the repeat count measures exactly one
        # kernel execution; the graded path builds with repeats=1.
        for _rep in range(repeats):
          # ---- input --------------------------------------------------
          x_fm = xpool.tile([P, ES, T], fp32, tag="x")
          nc.sync.dma_start(
            out=x_fm,
            in_=wf32.ap()[:, ds(0, ES * T)].rearrange("p (s t) -> p s t", s=ES))
          x_bf = xbfp.tile([P, ES, T], bf16, tag="xbf")
          nc.vector.tensor_copy(out=x_bf, in_=x_fm)

          for l in range(n_layers):
            # ===== QKV =====
            vb_t = wstream.tile([P, E], bf16, tag="vb", bufs=2)
            wdma(vb_t, bfw(f"vb{l}"))
            q_bf = qkp.tile([P, ES, T], bf16, tag="q")
            k_bf = qkp.tile([P, ES, T], bf16, tag="k")
            mark(f"qkv{l}")

            def qk_tile(m, wt_name, bias, dst):
                wt = wstream.tile([P, ES, P], bf16, tag="wqk", bufs=5, name="wt")
                wdma(wt, bfw(wt_name, ES * P, m * ES * P).rearrange(
                    "p (s q) -> p s q", s=ES))
                ps = ps_tile([P, T], "ps", 2)
                for s in range(ES):
                    nc.tensor.matmul(ps, lhsT=wt[:, s, :], rhs=x_bf[:, s, :],
                                     start=(s == 0), stop=(s == ES - 1))
                nc.vector.tensor_scalar_add(dst[:, m, :], ps, cref(bias, m))

            def v_chunk(ch):
                wvt = wstream.tile([P, ES, T], bf16, tag=f"wv{ch}", bufs=1,
                                   name="wvt")
                wdma(wvt, bfw(f"wv{l}", ES * T, ch * ES * T).rearrange(
                    "p (s t) -> p s t", s=ES))
                for tt in range(KT):
                    ps = ps_tile([P, T], "ps", 2)
                    for s in range(ES):
                        nc.tensor.matmul(ps, lhsT=x_bf[:, s, ts(tt, P)],
                                         rhs=wvt[:, s, :],
                                         start=(s == 0), stop=(s == ES - 1))
                    dst = v_aug[:, tt, ch * 8 : (ch + 1) * 8, 0:D]
                    nc.vector.tensor_add(
                        out=dst, in0=ps,
                        in1=vb_t[:, ds(ch * T, T)].rearrange(
                            "p (h d) -> p h d", d=D))

            def head_scores(h):
                # scores + exp + gaussian-bias mul for head h -> t2 tiles
                pb = (h % 2) * D
                sub = h // 2
                q_h = q_bf[pb : pb + D, sub, :]          # [64, 512]
                t2s = []
                for pair in range(2):
                    ps2 = ps_tile([P, 2, T], "ps2", 2)
                    for i in range(2):
                        kt = pair * 2 + i
                        nc.tensor.matmul(
                            ps2[:, i, :],
                            lhsT=k_bf[pb : pb + D, sub, ts(kt, P)],
                            rhs=q_h, start=True, stop=True)
                    e2 = et_p.tile([P, 2, T], bf16, tag="e", bufs=2, name="e2")
                    nc.scalar.activation(e2, ps2, AF.Exp)
                    t2 = et_p.tile([P, 2, T], bf16, tag="t", bufs=4, name="t2")
                    nc.vector.tensor_mul(
                        out=t2, in0=e2, in1=gbp[:, pair * 2 : pair * 2 + 2, :])
                    t2s.append(t2)
                return t2s

            def head_av(h, t2s):
                # attention-value matmuls + renormalization for head h
                pb = (h % 2) * D
                sub = h // 2
                ps_o = ps_tile([D + 1, T], "pso", 2)
                for kt in range(KT):
                    nc.tensor.matmul(ps_o, lhsT=v_aug[:, kt, h, :],
                                     rhs=t2s[kt // 2][:, kt % 2, :],
                                     start=(kt == 0), stop=(kt == KT - 1))
                # r = 1/st via exp(-ln); st = ps_o row 64
                ln_t = rows.tile([1, T], fp32, tag="row", name="lnr")
                nc.scalar.activation(ln_t, ps_o[D : D + 1, :], AF.Ln)
                r_row = rows.tile([1, T], fp32, tag="row", name="rr")
                nc.scalar.activation(r_row, ln_t, AF.Exp, scale=-1.0)
                rb = bc_p.tile([D, T], fp32, tag="rb", bufs=2)
                nc.gpsimd.partition_broadcast(rb, r_row)
                nc.vector.tensor_mul(out=o_cat[pb : pb + D, sub, :],
                                     in0=ps_o[0:D, :], in1=rb)

            def head_wave(hs):
                # software pipeline: emit head h+1's score matmuls before
                # head h's AV matmuls so the PE never waits on exp/mul
                prev = None
                for h in hs:
                    cur = (h, head_scores(h))
                    if prev is not None:
                        head_av(*prev)
                    prev = cur
                head_av(*prev)

            o_cat = ocat_p.tile([P, ES, T], bf16, tag="ocat")
            # Interleave: q/k m-tiles 0-3 + v chunk 0, then heads 0-7 run
            # their exps on ACT while PE moves on to m-tiles 4-7 + v chunk 1.
            for m in range(4):
                qk_tile(m, f"wq{l}", f"qb{l}", q_bf)
                qk_tile(m, f"wk{l}", f"kb{l}", k_bf)
            mark(f"v{l}")
            v_chunk(0)
            mark(f"attnA{l}")
            head_wave(range(8))
            for m in range(4, ES):
                qk_tile(m, f"wq{l}", f"qb{l}", q_bf)
                qk_tile(m, f"wk{l}", f"kb{l}", k_bf)
            v_chunk(1)
            mark(f"attnB{l}")
            head_wave(range(8, H))
            # ===== out-proj + residual =====
            mark(f"outproj{l}")
            z1 = xpool.tile([P, ES, T], fp32, tag="x")
            for m in range(ES):
                wt = wstream.tile([P, ES, P], bf16, tag="wqk", bufs=5)
                wdma(wt, bfw(f"wo{l}", ES * P, m * ES * P).rearrange(
                    "p (s q) -> p s q", s=ES))
                ps = ps_tile([P, T], "ps", 2)
                for s in range(ES):
                    nc.tensor.matmul(ps, lhsT=wt[:, s, :], rhs=o_cat[:, s, :],
                                     start=(s == 0), stop=(s == ES - 1))
                po = small.tile([P, T], fp32, tag="po")
                nc.scalar.activation(po, ps, AF.Identity,
                                     bias=cref(f"ob{l}", m))
                nc.vector.tensor_add(out=z1[:, m, :], in0=x_fm[:, m, :], in1=po)

            def layernorm(z, g_name, b_name, want_bf=True):
                # stats: per-subtile cast + square feeding ones-matmuls, so
                # dependencies stay subtile-granular
                ps_st = ps_tile([33, T], "ps", 2)
                ps_s1 = ps_st[0:1, :]
                ps_s2 = ps_st[32:33, :]
                for s in range(ES):
                    zb = small.tile([P, T], bf16, tag="zb", bufs=3, name="zb")
                    nc.vector.tensor_copy(out=zb, in_=z[:, s, :])
                    # start=True only on the very first matmul: it clears the
                    # whole bank; row 1's first write lands on cleared
                    # has_written bits and overwrites, later ones accumulate.
                    nc.tensor.matmul(ps_s1, lhsT=ones_bf, rhs=zb,
                                     start=(s == 0), stop=(s == ES - 1))
                    zq = small.tile([P, T], bf16, tag="zq", name="zq")
                    nc.vector.tensor_mul(out=zq, in0=z[:, s, :], in1=z[:, s, :])
                    nc.tensor.matmul(ps_s2, lhsT=ones_bf, rhs=zq,
                                     start=False, stop=(s == ES - 1))
                mu = rows.tile([1, T], fp32, tag="row", name="mu")
                nc.vector.tensor_scalar_mul(mu, ps_s1, 1.0 / E)
                m2 = rows.tile([1, T], fp32, tag="row", name="m2")
                nc.vector.tensor_scalar_mul(m2, ps_s2, 1.0 / E)
                musq = rows.tile([1, T], fp32, tag="row", name="musq")
                nc.scalar.activation(musq, mu, AF.Square)
                var = rows.tile([1, T], fp32, tag="row", name="var")
                nc.vector.tensor_sub(out=var, in0=m2, in1=musq)
                lnv = rows.tile([1, T], fp32, tag="row", name="lnv")
                nc.scalar.activation(lnv, var, AF.Ln, bias=eps_row)
                rstd = rows.tile([1, T], fp32, tag="row", name="rstd")
                nc.scalar.activation(rstd, lnv, AF.Exp, scale=-0.5)
                crow = rows.tile([1, T], fp32, tag="row", name="crow")
                nc.vector.tensor_mul(out=crow, in0=mu, in1=rstd)
                ab = bc_p.tile([P, T], fp32, tag="ab")
                nc.gpsimd.partition_broadcast(ab, rstd)
                cb = bc_p.tile([P, T], fp32, tag="cb")
                nc.gpsimd.partition_broadcast(cb, crow)
                out_fm = xpool.tile([P, ES, T], fp32, tag="x")
                out_bf = xbfp.tile([P, ES, T], bf16, tag="xbf", name="out_bf") if want_bf else None
                for s in range(ES):
                    t1 = small.tile([P, T], fp32, tag="t1")
                    nc.vector.tensor_mul(out=t1, in0=z[:, s, :], in1=ab)
                    t2 = small.tile([P, T], fp32, tag="t2")
                    nc.vector.tensor_sub(out=t2, in0=t1, in1=cb)
                    # bf16 result first (it gates the next layer's matmuls);
                    # the fp32 residual copy is off the critical path
                    if want_bf:
                        nc.vector.tensor_scalar(
                            out_bf[:, s, :], t2,
                            scalar1=cref(g_name, s), scalar2=cref(b_name, s),
                            op0=OP.mult, op1=OP.add)
                    nc.vector.tensor_scalar(
                        out_fm[:, s, :], t2,
                        scalar1=cref(g_name, s), scalar2=cref(b_name, s),
                        op0=OP.mult, op1=OP.add)
                return out_fm, out_bf

            mark(f"ln1_{l}")
            x_fm, x_bf = layernorm(z1, f"g1{l}", f"b1{l}")

            # ===== FFN =====
            mark(f"ffn1{l}")
            h1 = h1p.tile([P, FS, T], bf16, tag="h1")
            for f in range(FS):
                wt = wstream.tile([P, ES, P], bf16, tag="wqk", bufs=5)
                wdma(wt, bfw(f"w1{l}", ES * P, f * ES * P).rearrange(
                    "p (s q) -> p s q", s=ES))
                ps = ps_tile([P, T], "ps", 2)
                for s in range(ES):
                    nc.tensor.matmul(ps, lhsT=wt[:, s, :], rhs=x_bf[:, s, :],
                                     start=(s == 0), stop=(s == ES - 1))
                nc.scalar.activation(h1[:, f, :], ps, AF.Relu,
                                     bias=cref(f"f1b{l}", f))
            mark(f"ffn2{l}")
            z2 = xpool.tile([P, ES, T], fp32, tag="x")
            for m in range(ES):
                ps = ps_tile([P, T], "ps", 2)
                for chk in range(2):
                    wt = wstream.tile([P, FS // 2, P], bf16, tag="w2", bufs=4)
                    wdma(wt, bfw(f"w2{l}", (FS // 2) * P,
                                 (m * 2 + chk) * (FS // 2) * P).rearrange(
                        "p (f q) -> p f q", f=FS // 2))
                    for fs in range(FS // 2):
                        nc.tensor.matmul(
                            ps, lhsT=wt[:, fs, :], rhs=h1[:, chk * 16 + fs, :],
                            start=(chk == 0 and fs == 0),
                            stop=(chk == 1 and fs == FS // 2 - 1))
                po = small.tile([P, T], fp32, tag="po")
                nc.scalar.activation(po, ps, AF.Identity,
                                     bias=cref(f"f2b{l}", m))
                nc.vector.tensor_add(out=z2[:, m, :], in0=x_fm[:, m, :], in1=po)
            mark(f"ln2_{l}")
            _want = l != n_layers - 1
            x_fm, x_bf2 = layernorm(z2, f"g2{l}", f"b2{l}", want_bf=_want)
            if _want:
                x_bf = x_bf2

          mark("head")
          # ---- head: y = x[last] . hw + hb ----------------------------
          xl = small.tile([P, ES, 1], fp32, tag="xl")
          nc.vector.tensor_mul(out=xl, in0=x_fm[:, :, T - 1 : T],
                               in1=hw_sb[:, :, None])
          xl_r = small.tile([P, 1], fp32, tag="xlr")
          nc.vector.reduce_sum(xl_r, xl, axis=mybir.AxisListType.XYZW)
          xl_bf = small.tile([P, 1], bf16, tag="xlbf")
          nc.vector.tensor_copy(out=xl_bf, in_=xl_r)
          ps_y = ps_tile([1, 1], "ps", 2)
          nc.tensor.matmul(ps_y, lhsT=ones_bf, rhs=xl_bf, start=True, stop=True)
          y_sb = small.tile([1, 1], fp32, tag="ysb")
          nc.vector.tensor_add(out=y_sb, in0=ps_y, in1=hb_sb)
          nc.sync.dma_start(out=y_out.ap(), in_=y_sb)

    nc.finalize()
    return nc


def _host_prep(inputs, n_layers=L):
    """Build the per-core input maps: two packed blobs in exact SBUF layouts."""
    f32 = np.float32
    LBF, LF32 = _layout(n_layers)
    CBF, CF32 = LBF["_total"], LF32["_total"]

    def fm(a2d):  # [rows, cols] -> partition-major [P, rows//P, cols]
        rows, cols = a2d.shape
        return np.ascontiguousarray(
            a2d.reshape(rows // P, P, cols).transpose(1, 0, 2))

    def mtiled(a2d):  # lhsT [K, M] -> [M//P, P, K//P, P] per-m-tile contiguous
        arr = fm(a2d)  # [P, K//P, M]
        ksub = arr.shape[1]
        m_t = arr.shape[2] // P
        return np.ascontiguousarray(
            arr.reshape(P, ksub, m_t, P).transpose(2, 0, 1, 3))

    def col(a1d):  # [rows] -> per-partition [P, rows//P]
        return np.ascontiguousarray(a1d.reshape(-1, P).T)

    src = np.asarray(inputs["src"], f32)
    pos = np.arange(B, dtype=f32)[:, None]
    div = np.exp(np.arange(0, E, 2, dtype=f32) * (-math.log(10000.0) / E))
    pe = np.zeros((B, E), f32)
    pe[:, 0::2] = np.sin(pos * div)
    pe[:, 1::2] = np.cos(pos * div)
    x0 = src + pe[:, None, :]  # [B, T, E]

    dist = np.arange(T, dtype=f32)[:, None] - np.arange(T, dtype=f32)[None, :]
    gb = np.exp(-(dist**2) / (2.0 * (T / 4.0) ** 2)) + 1e-5

    wbf = np.empty((P, CBF), BF16)
    wf32 = np.empty((P, CF32), f32)

    def put_bf(name, arr_pm):  # arr_pm: [P, ncols]
        off, n = LBF[name]
        assert arr_pm.shape == (P, n), (name, arr_pm.shape, n)
        wbf[:, off : off + n] = arr_pm

    def put_f32(name, arr_pm):
        off, n = LF32[name]
        assert arr_pm.shape == (P, n), (name, arr_pm.shape, n)
        wf32[:, off : off + n] = arr_pm

    put_bf("gbp", fm(gb).reshape(P, -1).astype(BF16))

    ipw = np.asarray(inputs["in_proj_w"], f32)
    ipb = np.asarray(inputs["in_proj_b"], f32)
    out_w = np.asarray(inputs["out_w"], f32)
    out_b = np.asarray(inputs["out_b"], f32)
    ff1_w = np.asarray(inputs["ff1_w"], f32)
    ff1_b = np.asarray(inputs["ff1_b"], f32)
    ff2_w = np.asarray(inputs["ff2_w"], f32)
    ff2_b = np.asarray(inputs["ff2_b"], f32)
    ln1_g = np.asarray(inputs["ln1_g"], f32)
    ln1_b = np.asarray(inputs["ln1_b"], f32)
    ln2_g = np.asarray(inputs["ln2_g"], f32)
    ln2_b = np.asarray(inputs["ln2_b"], f32)

    def tiles_pm(mt):  # [m_t, P, ksub, Pq] -> [P, m_t*ksub*Pq] (m-major cols)
        return np.ascontiguousarray(
            mt.transpose(1, 0, 2, 3)).reshape(P, -1)

    for l in range(n_layers):
        put_bf(f"wq{l}", tiles_pm(mtiled((ipw[l, 0:E] / 8.0).T).astype(BF16)))
        put_bf(f"wk{l}", tiles_pm(mtiled(ipw[l, E : 2 * E].T).astype(BF16)))
        wv_fm = fm(ipw[l, 2 * E : 3 * E].T)  # [P, ES, E]
        wv_ch = np.ascontiguousarray(
            wv_fm.reshape(P, ES, 2, T).transpose(2, 0, 1, 3)).astype(BF16)
        put_bf(f"wv{l}", wv_ch.transpose(1, 0, 2, 3).reshape(P, -1))
        put_bf(f"wo{l}", tiles_pm(mtiled(out_w[l].T).astype(BF16)))
        put_bf(f"w1{l}", tiles_pm(mtiled(ff1_w[l].T).astype(BF16)))
        w2t = mtiled(ff2_w[l].T)  # [ES, P, FS, P]
        w2c = np.ascontiguousarray(
            w2t.reshape(ES, P, 2, FS // 2, P).transpose(0, 2, 1, 3, 4)).astype(BF16)
        # [ES, 2, P, FS//2, P] -> cols (m, chk) major
        put_bf(f"w2{l}", np.ascontiguousarray(
            w2c.transpose(2, 0, 1, 3, 4)).reshape(P, -1))
        put_bf(f"vb{l}", np.broadcast_to(
            ipb[l, 2 * E : 3 * E].astype(BF16), (P, E)))
        put_f32(f"qb{l}", col(ipb[l, 0:E] / 8.0))
        put_f32(f"kb{l}", col(ipb[l, E : 2 * E]))
        put_f32(f"ob{l}", col(out_b[l]))
        put_f32(f"f1b{l}", col(ff1_b[l]))
        put_f32(f"f2b{l}", col(ff2_b[l]))
        put_f32(f"g1{l}", col(ln1_g[l]))
        put_f32(f"b1{l}", col(ln1_b[l]))
        put_f32(f"g2{l}", col(ln2_g[l]))
        put_f32(f"b2{l}", col(ln2_b[l]))
    put_f32("hw", col(np.asarray(inputs["head_w"], f32)[0]))
    put_f32("hb", np.full((P, 1), np.asarray(inputs["head_b"], f32).ravel()[0],
                          f32))

    in_maps = []
    x0_off = LF32["x0"][0]
    for c in range(B):
        m = dict(wbf=wbf)
        mf = wf32.copy()
        mf[:, x0_off : x0_off + ES * T] = fm(x0[c].T).reshape(P, -1)
        m["wf32"] = mf
        in_maps.append(m)
    return in_maps


def kernel(**inputs):
    from concourse.bass_utils import run_bass_kernel_spmd

    if "nc" not in _CACHE:
        _CACHE["nc"] = _build_bass()
    nc = _CACHE["nc"]
    in_maps = _host_prep(inputs)
    res = run_bass_kernel_spmd(nc, in_maps, core_ids=list(range(B)))
    y = np.stack([res.results[c]["y"].reshape(1) for c in range(B)], axis=0)
    return y.astype(np.float32)
